# revision 1
# baseline (speedup 1.0000x reference)
"""SGConv (K=2) GNN message-passing kernel for Trainium2 (8 NeuronCores), v3.

out = (D^{-1/2} (A+I) D^{-1/2})^2 @ x @ W.T

v3 = the proven dma_gather kernel (v1) plus:
  - Host-side balanced node placement: nodes are assigned to
    (core, tile, slot) so that (a) per-core edge counts are equal,
    (b) the A/B table halves carry equal source mass, and (c) every
    (core, tile) needs ~3 A-chunks and ~3 B-chunks -> ~300 gather chunks
    per hop instead of ~420 (28% fewer SWDGE descriptor-gen rows, the
    measured bottleneck).
  - Self-loops removed from the gather stream; each output tile adds its
    local shard tile via an identity matmul (locals stay SBUF-resident).
  - Symmetric-norm folding: table rows are pre-scaled by dinv at
    projection; hop outputs are row-scaled by dinv^2 (hop1) / dinv
    (hop2) on the Activation engine. No per-edge norm multiplies or norm
    tables.
  - Gather batches of 14 chunks (1792 idxs, 113 ring descriptors).

Self-contained: hardcodes NCORES=8; shapes derived from inputs.
"""

import os
import heapq
import numpy as np
import ml_dtypes

from concourse import bacc, mybir, tile
from concourse.bass_utils import run_bass_kernel_spmd

NCORES = 8
P = 128
F32 = mybir.dt.float32
BF16 = mybir.dt.bfloat16
I16 = mybir.dt.int16

SEG_CHUNKS = int(os.environ.get("GNN_SEG", "8"))

LAST_RESULTS = None


def _ceil(a, b):
    return -(-a // b)


def _wrap_idx(idx):
    """int16 [n] -> dma_gather layout [128, n//16]."""
    n = idx.shape[0]
    assert n % 16 == 0
    w = np.ascontiguousarray(idx.reshape(n // 16, 16).T).astype(np.int16)
    return np.ascontiguousarray(np.tile(w, (8, 1)))


def _balance_1d(nodes, weights, nbins, caps):
    """Greedy 1-D: sort by weight desc, assign to lightest bin with space."""
    order = nodes[np.argsort(-weights[nodes], kind="stable")]
    bin_of = np.full(len(weights), -1, np.int32)
    cnt = [0] * nbins
    load = [0.0] * nbins
    heap = [(0.0, b) for b in range(nbins)]
    heapq.heapify(heap)
    for v in order:
        spill = []
        while True:
            w, b = heapq.heappop(heap)
            if cnt[b] < caps[b]:
                break
            spill.append((w, b))
        bin_of[v] = b
        cnt[b] += 1
        load[b] += weights[v]
        heapq.heappush(heap, (load[b], b))
        for it in spill:
            heapq.heappush(heap, it)
    return bin_of


def _balance_2d(nodes, wa, wb, nbins, caps, cap_a, cap_b):
    """Greedy 2-D: sort by total desc; place in the bin (with node space)
    that minimizes the resulting max(A/capA, B/capB). Then a swap-repair
    pass pulls every bin under (cap_a, cap_b) where feasible."""
    order = nodes[np.argsort(-(wa[nodes] + wb[nodes]), kind="stable")]
    bin_of = np.full(len(wa), -1, np.int32)
    cnt = np.zeros(nbins, int)
    la = np.zeros(nbins)
    lb = np.zeros(nbins)
    for v in order:
        cand = np.nonzero(cnt < caps)[0]
        cost = np.maximum((la[cand] + wa[v]) / cap_a,
                          (lb[cand] + wb[v]) / cap_b)
        b = cand[np.argmin(cost)]
        bin_of[v] = b
        cnt[b] += 1
        la[b] += wa[v]
        lb[b] += wb[v]

    members = [list(nodes[bin_of[nodes] == t]) for t in range(nbins)]

    def viol(a, b):
        return np.maximum(0.0, a - cap_a) + np.maximum(0.0, b - cap_b)

    def viol_v(a, b):
        return np.maximum(0.0, a - cap_a) + np.maximum(0.0, b - cap_b)

    for _sweep in range(60):
        improved = False
        vl = viol_v(la, lb)
        for t in np.argsort(-vl):
            t = int(t)
            if vl[t] <= 0:
                break
            best = None
            mu = np.asarray(members[t])
            for s in range(nbins):
                if s == t or not members[s]:
                    continue
                mv = np.asarray(members[s])
                da = wa[mu][:, None] - wa[mv][None, :]
                db = wb[mu][:, None] - wb[mv][None, :]
                nv = (viol_v(la[t] - da, lb[t] - db)
                      + viol_v(la[s] + da, lb[s] + db))
                gain = (vl[t] + vl[s]) - nv
                j = int(np.argmax(gain))
                gj = float(gain.ravel()[j])
                if gj > 1e-9 and (best is None or gj > best[0]):
                    ui, vi = np.unravel_index(j, gain.shape)
                    best = (gj, int(mu[ui]), int(mv[vi]), s)
            if best is None:
                continue
            _, u, v, s = best
            members[t].remove(u)
            members[s].remove(v)
            members[t].append(v)
            members[s].append(u)
            bin_of[u] = s
            bin_of[v] = t
            la[t] += wa[v] - wa[u]
            lb[t] += wb[v] - wb[u]
            la[s] += wa[u] - wa[v]
            lb[s] += wb[u] - wb[v]
            vl = viol_v(la, lb)
            improved = True
        if not improved or viol_v(la, lb).sum() <= 0:
            break
    return bin_of


def _prepare(x, edge_index, W):
    x = np.ascontiguousarray(np.asarray(x, dtype=np.float32))
    W = np.ascontiguousarray(np.asarray(W, dtype=np.float32))
    ei = np.asarray(edge_index).astype(np.int64)

    N, Din = x.shape
    Dout = int(W.shape[0])
    assert N % NCORES == 0
    PN = N // NCORES
    T = _ceil(PN, P)
    PNp = T * P
    KT = Din // P
    TS = T // 2 + 1          # 25 tiles in part A
    RA = TS * P              # 3200
    RB = PNp - RA            # 3072
    assert NCORES * RA < 2**15 and NCORES * RB < 2**15

    indeg = np.bincount(ei[1], minlength=N).astype(np.float64)
    outdeg = np.bincount(ei[0], minlength=N).astype(np.float64)
    dinv = (1.0 / np.sqrt(indeg + 1.0)).astype(np.float32)

    allv = np.arange(N)
    core_of = _balance_1d(allv, indeg, NCORES, [PN] * NCORES)

    # Split each core's nodes into halves A (RA slots) / B, equalizing
    # OUT-degree mass so A/B-sourced edge counts match; preserve IN-degree
    # balance by swapping only equal-indeg pairs.
    halfA = np.zeros(N, bool)
    capsA = [P] * TS
    capsB = [P] * (T - TS)
    capsB[-1] = PN - RA - (T - TS - 1) * P   # last tile short
    nA = sum(capsA)
    for c in range(NCORES):
        nodes = allv[core_of == c]
        so = nodes[np.argsort(-outdeg[nodes], kind="stable")]
        sel = np.zeros(len(so), bool)
        sel[0::2] = True
        diff = int(sel.sum()) - nA
        if diff > 0:
            sel[np.nonzero(sel)[0][-diff:]] = False
        elif diff < 0:
            sel[np.nonzero(~sel)[0][diff:]] = True
        A = list(so[sel])
        Bn = list(so[~sel])

        def _mass_swap(A, Bn, keyd, vald, need):
            """Swap equal-keyd pairs to move ~need of vald mass into A."""
            if abs(need) <= 1:
                return
            byd_A, byd_B = {}, {}
            for i, v in enumerate(A):
                byd_A.setdefault(int(keyd[v]), []).append(i)
            for i, v in enumerate(Bn):
                byd_B.setdefault(int(keyd[v]), []).append(i)
            sgn = 1.0 if need > 0 else -1.0
            pairs = []
            for d in byd_A:
                if d not in byd_B:
                    continue
                ai = sorted(byd_A[d], key=lambda i: sgn * vald[A[i]])
                bi = sorted(byd_B[d], key=lambda i: -sgn * vald[Bn[i]])
                for i, j in zip(ai, bi):
                    pairs.append((vald[Bn[j]] - vald[A[i]], i, j))
            pairs.sort(key=lambda t: -sgn * t[0])
            acc = 0.0
            for gain, i, j in pairs:
                if abs(acc) >= abs(need) or sgn * gain <= 0:
                    break
                A[i], Bn[j] = Bn[j], A[i]
                acc += gain

        # per-bin in-mass parity: A holds TS/T of the in-degree mass
        tgt_in = (indeg[A].sum() + indeg[Bn].sum()) * TS / T
        _mass_swap(A, Bn, outdeg, indeg, tgt_in - indeg[A].sum())
        # out-mass parity via equal-indeg swaps (keeps in-mass fixed)
        _mass_swap(A, Bn, indeg, outdeg,
                   (outdeg[Bn].sum() - outdeg[A].sum()) / 2.0)
        halfA[np.asarray(A, dtype=np.int64)] = True

    # edge source-part weights, then per-core skew fix: the A-half's bins
    # can absorb only TS*384 of wa-mass, so Sum_{v in A}(wa-wb) must be
    # ~0. Swap equal-(indeg,outdeg) pairs across halves (preserves the
    # mass balances; only flips source parts of the pair's out-edges).
    def _wab():
        ewa = halfA[ei[0]]
        wa = np.zeros(N)
        wb = np.zeros(N)
        np.add.at(wa, ei[1][ewa], 1)
        np.add.at(wb, ei[1][~ewa], 1)
        return wa, wb

    # Global pass: each core's incoming source-part skew
    # skew_c = (#A-sourced - #B-sourced edges into core c) must stay small
    # (|skew_c| <= ~120 for per-half feasibility). Flip halves of
    # equal-(indeg,outdeg) node pairs (same owner core) chosen by their
    # out-edge target profiles to minimize ||skewvec||.
    M = np.zeros((N, NCORES), np.float64)
    np.add.at(M, (ei[0], core_of[ei[1]]), 1.0)
    sv = (M[halfA].sum(axis=0) - M[~halfA].sum(axis=0))
    rng = np.random.default_rng(0)
    key2 = (core_of.astype(np.int64) * 100000
            + indeg.astype(np.int64) * 64 + outdeg.astype(np.int64))
    cls = {}
    for v in allv:
        cls.setdefault(int(key2[v]), [[], []])[0 if halfA[v] else 1].append(
            int(v))
    cls_keys = [k for k, ab in cls.items() if ab[0] and ab[1]]
    for _ in range(4000):
        if np.abs(sv).max() <= 48:
            break
        best = None
        for _try in range(64):
            k = cls_keys[int(rng.integers(len(cls_keys)))]
            a_list, b_list = cls[k]
            if not a_list or not b_list:
                continue
            u = a_list[int(rng.integers(len(a_list)))]
            v = b_list[int(rng.integers(len(b_list)))]
            d = 2.0 * (M[v] - M[u])
            gain = float(np.abs(sv).sum() - np.abs(sv + d).sum())
            if gain > 0 and (best is None or gain > best[0]):
                best = (gain, u, v, k, d)
        if best is None:
            continue
        _, u, v, k, d = best
        cls[k][0].remove(u)
        cls[k][1].remove(v)
        cls[k][0].append(v)
        cls[k][1].append(u)
        halfA[u] = False
        halfA[v] = True
        sv = sv + d

    wa, wb = _wab()
    skew = wa - wb
    for c in range(NCORES):
        nodes = allv[core_of == c]
        A = list(nodes[halfA[nodes]])
        Bn = list(nodes[~halfA[nodes]])
        need = -skew[A].sum() / 2.0   # want Sum_A skew -> 0
        if abs(need) > 2:
            key2 = (indeg * 64 + outdeg).astype(np.int64)
            _mass_swap2 = []
            byd_A, byd_B = {}, {}
            for i, v in enumerate(A):
                byd_A.setdefault(int(key2[v]), []).append(i)
            for i, v in enumerate(Bn):
                byd_B.setdefault(int(key2[v]), []).append(i)
            sgn = 1.0 if need > 0 else -1.0
            pairs = []
            for d in byd_A:
                if d not in byd_B:
                    continue
                ai = sorted(byd_A[d], key=lambda i: sgn * skew[A[i]])
                bi = sorted(byd_B[d], key=lambda i: -sgn * skew[Bn[i]])
                for i, j in zip(ai, bi):
                    pairs.append((skew[Bn[j]] - skew[A[i]], i, j))
            pairs.sort(key=lambda t: -sgn * t[0])
            acc = 0.0
            for gain, i, j in pairs:
                if abs(acc) >= abs(need) or sgn * gain <= 0:
                    break
                u, v = A[i], Bn[j]
                A[i], Bn[j] = v, u
                halfA[u] = False
                halfA[v] = True
                acc += gain
    wa, wb = _wab()

    tile_of = np.zeros(N, np.int32)
    slot_of = np.zeros(N, np.int32)
    for c in range(NCORES):
        nodes = allv[core_of == c]
        A = nodes[halfA[nodes]]
        Bn = nodes[~halfA[nodes]]
        tb = _balance_2d(A, wa, wb, TS, np.asarray(capsA), 384.0, 384.0)
        tile_of[A] = tb[A]
        for t in range(TS):
            sel = A[tb[A] == t]
            slot_of[sel] = np.arange(len(sel))
        tb2 = _balance_2d(Bn, wa, wb, T - TS, np.asarray(capsB), 384.0,
                          384.0)
        tile_of[Bn] = TS + tb2[Bn]
        for t in range(T - TS):
            sel = Bn[tb2[Bn] == t]
            slot_of[sel] = np.arange(len(sel))

    # Relabel tiles per core (within each half): bins that overflow the
    # A-chunk budget go to the FRONT, B-overflow bins to the BACK, so the
    # cross-core max pays each overflow category once (aligned) instead of
    # once per scattered tile.
    for c in range(NCORES):
        nodes = allv[core_of == c]
        for lo, hi in ((0, TS), (TS, T)):
            keys = []
            for t in range(lo, hi):
                m = nodes[tile_of[nodes] == t]
                ao = _ceil(int(wa[m].sum()), P) > 3
                bo = _ceil(int(wb[m].sum()), P) > 3
                # A-over (incl both-over) front desc; B-over back;
                # normals in the middle
                if ao:
                    k = -2000 - int(wa[m].sum())
                elif bo:
                    k = 2000 + int(wb[m].sum())
                else:
                    k = -int(wa[m].sum() + wb[m].sum())
                keys.append(k)
            order2 = sorted(range(hi - lo), key=lambda i: keys[i])
            remap = np.zeros(hi - lo, np.int32)
            for newt, oldt in enumerate(order2):
                remap[oldt] = newt
            sel = nodes[(tile_of[nodes] >= lo) & (tile_of[nodes] < hi)]
            tile_of[sel] = lo + remap[tile_of[sel] - lo]

    pos = (core_of.astype(np.int64) * PNp + tile_of * P + slot_of)

    # ---- edge lists (no self-loops) ----
    dst_core = core_of[ei[1]]
    dst_tile = tile_of[ei[1]]
    dstloc = slot_of[ei[1]].astype(np.float32)
    s_pos = pos[ei[0]]
    s_core = s_pos // PNp
    s_off = s_pos % PNp
    part = (s_off >= RA).astype(np.int64)
    srcloc = np.where(part == 1,
                      s_core * RB + (s_off - RA),
                      s_core * RA + s_off)

    key = (dst_core * T + dst_tile) * 2 + part
    order = np.argsort(key, kind="stable")
    s_srcloc = srcloc[order]
    s_dstloc = dstloc[order]

    cnt = np.zeros(NCORES * T * 2, np.int64)
    np.add.at(cnt, key, 1)
    cnt = cnt.reshape(NCORES, T, 2)
    nch = _ceil(cnt, P)
    nch_u = nch.max(axis=0)            # [T, 2]
    NL = int(nch_u[:, 0].sum())
    NH = int(nch_u[:, 1].sum())

    starts = np.zeros(NCORES * T * 2 + 1, np.int64)
    starts[1:] = np.cumsum(cnt.reshape(-1))

    iota = np.ascontiguousarray(
        np.tile(np.arange(P, dtype=np.float32), (P, 1)))
    ident = np.ascontiguousarray(np.eye(P, dtype=np.float32))
    wt = np.ascontiguousarray(
        W.T.reshape(KT, P, Dout).transpose(1, 0, 2).reshape(P, KT * Dout))

    dinv_p = np.zeros(NCORES * PNp, np.float32)
    dinv_p[pos] = dinv

    in_maps = []
    for c in range(NCORES):
        idx_f = [np.zeros(NL * P, np.int64), np.zeros(NH * P, np.int64)]
        dl_f = [np.full(NL * P, -1.0, np.float32),
                np.full(NH * P, -1.0, np.float32)]
        off = [0, 0]
        for t in range(T):
            for h in (0, 1):
                k = (c * T + t) * 2 + h
                a, b = int(starts[k]), int(starts[k + 1])
                n = b - a
                o = off[h] * P
                idx_f[h][o:o + n] = s_srcloc[a:b]
                dl_f[h][o:o + n] = s_dstloc[a:b]
                off[h] += int(nch_u[t, h])
        assert off[0] == NL and off[1] == NH

        mine = np.nonzero(core_of == c)[0]
        xp = np.zeros((PNp, Din), np.float32)
        xp[tile_of[mine] * P + slot_of[mine]] = x[mine]
        xt = np.ascontiguousarray(
            xp.T.reshape(KT, P, PNp).transpose(1, 0, 2).reshape(
                P, KT * PNp).astype(ml_dtypes.bfloat16))

        dv = np.ascontiguousarray(
            dinv_p[c * PNp:(c + 1) * PNp].reshape(T, P).T)  # [P, T]
        scl = np.ascontiguousarray(
            np.concatenate([dv, dv * dv, dv], axis=1))      # [P, 3T]

        in_maps.append({
            "xt": xt,
            "wt": np.ascontiguousarray(wt.astype(ml_dtypes.bfloat16)),
            "iota": iota,
            "ident": ident,
            "scl": scl,
            "idxlo": _wrap_idx(idx_f[0].astype(np.int16)),
            "idxhi": _wrap_idx(idx_f[1].astype(np.int16)),
            "dllo": np.ascontiguousarray(dl_f[0].reshape(NL, P).T),
            "dlhi": np.ascontiguousarray(dl_f[1].reshape(NH, P).T),
        })

    dims = dict(N=N, PNp=PNp, T=T, KT=KT, Din=Din, Dout=Dout, TS=TS,
                RA=RA, RB=RB)
    return dims, nch_u, in_maps, pos


def _build(dims, nch_u):
    PNp, T, KT, Dout = dims["PNp"], dims["T"], dims["KT"], dims["Dout"]
    TS, RA, RB = dims["TS"], dims["RA"], dims["RB"]
    NL = int(nch_u[:, 0].sum())
    NH = int(nch_u[:, 1].sum())

    nc = bacc.Bacc("TRN2", target_bir_lowering=False, debug=False,
                   num_devices=NCORES)

    xt_d = nc.dram_tensor("xt", [P, KT * PNp], BF16,
                      kind="ExternalInput")
    wt_d = nc.dram_tensor("wt", [P, KT * Dout], BF16,
                          kind="ExternalInput")
    iota_d = nc.dram_tensor("iota", [P, P], F32, kind="ExternalInput")
    id_d = nc.dram_tensor("ident", [P, P], F32, kind="ExternalInput")
    scl_d = nc.dram_tensor("scl", [P, 3 * T], F32, kind="ExternalInput")
    idxlo_d = nc.dram_tensor("idxlo", [P, NL * 8], I16, kind="ExternalInput")
    idxhi_d = nc.dram_tensor("idxhi", [P, NH * 8], I16, kind="ExternalInput")
    dllo_d = nc.dram_tensor("dllo", [P, NL], F32, kind="ExternalInput")
    dlhi_d = nc.dram_tensor("dlhi", [P, NH], F32, kind="ExternalInput")
    out_d = nc.dram_tensor("out", [PNp, Dout], F32, kind="ExternalOutput")

    h0sA = nc.dram_tensor("h0sA", [RA, Dout], F32)
    h0sB = nc.dram_tensor("h0sB", [RB, Dout], F32)
    h0fA = nc.dram_tensor("h0fA", [NCORES * RA, Dout], F32,
                          addr_space="Shared")
    h0fB = nc.dram_tensor("h0fB", [NCORES * RB, Dout], F32,
                          addr_space="Shared")
    h1sA = nc.dram_tensor("h1sA", [RA, Dout], F32)
    h1sB = nc.dram_tensor("h1sB", [RB, Dout], F32)
    h1fA = nc.dram_tensor("h1fA", [NCORES * RA, Dout], F32,
                          addr_space="Shared")
    h1fB = nc.dram_tensor("h1fB", [NCORES * RB, Dout], F32,
                          addr_space="Shared")

    rg = [list(range(NCORES))]

    def allgather(src, dst):
        nc.gpsimd.collective_compute(
            "AllGather", mybir.AluOpType.bypass, replica_groups=rg,
            ins=[src.ap().opt()], outs=[dst.ap().opt()])

    with tile.TileContext(nc) as tc:
        with tc.tile_pool(name="const", bufs=1) as constp:
            wts = constp.tile([P, KT * Dout], BF16)
            nc.sync.dma_start(out=wts[:], in_=wt_d[:, :])
            iota_t = constp.tile([P, P], F32)
            nc.sync.dma_start(out=iota_t[:], in_=iota_d[:, :])
            ident = constp.tile([P, P], F32)
            nc.sync.dma_start(out=ident[:], in_=id_d[:, :])
            scl_t = constp.tile([P, 3 * T], F32)
            nc.sync.dma_start(out=scl_t[:], in_=scl_d[:, :])
            idxlo_t = constp.tile([P, NL * 8], I16)
            nc.sync.dma_start(out=idxlo_t[:], in_=idxlo_d[:, :])
            idxhi_t = constp.tile([P, NH * 8], I16)
            nc.sync.dma_start(out=idxhi_t[:], in_=idxhi_d[:, :])
            dllo_t = constp.tile([P, NL], F32)
            nc.sync.dma_start(out=dllo_t[:], in_=dllo_d[:, :])
            dlhi_t = constp.tile([P, NH], F32)
            nc.sync.dma_start(out=dlhi_t[:], in_=dlhi_d[:, :])

            # hop-input locals (row-major, dinv-scaled), SBUF-resident
            tloc = [constp.tile([P, T * Dout], F32, tag=f"tloc{k}",
                                name=f"tloc{k}")
                    for k in range(2)]

            # ---------------- projection: t0 = dinv * (x @ W.T) ----------
            with tc.tile_pool(name="proj", bufs=1) as projp, \
                 tc.tile_pool(name="ppsum", bufs=4, space="PSUM") as ppsum:
                xts = projp.tile([P, KT * PNp], BF16)
                for k in range(KT):
                    for lo, ln in ((0, RA), (RA, RB)):
                        nc.sync.dma_start(
                            out=xts[:, k * PNp + lo:k * PNp + lo + ln],
                            in_=xt_d[:, k * PNp + lo:k * PNp + lo + ln])
                for m in range(T):
                    ps = ppsum.tile([P, Dout], F32)
                    for k in range(KT):
                        nc.tensor.matmul(
                            out=ps[:, :],
                            lhsT=xts[:,
                                     k * PNp + m * P:k * PNp + (m + 1) * P],
                            rhs=wts[:, k * Dout:(k + 1) * Dout],
                            start=(k == 0), stop=(k == KT - 1))
                    dstv = tloc[0][:, m * Dout:(m + 1) * Dout]
                    nc.vector.tensor_tensor(
                        out=dstv, in0=ps[:, :],
                        in1=scl_t[:, m:m + 1].broadcast_to([P, Dout]),
                        op=mybir.AluOpType.mult)
                    if m < TS:
                        nc.sync.dma_start(
                            out=h0sA[m * P:(m + 1) * P, :], in_=dstv)
                    else:
                        nc.sync.dma_start(
                            out=h0sB[m * P - RA:(m + 1) * P - RA, :],
                            in_=dstv)
                    if m == TS - 1:
                        allgather(h0sA, h0fA)
                allgather(h0sB, h0fB)

            def hop(kk, tblA, tblB, dst_write):
                with tc.tile_pool(name=f"vals{kk}", bufs=1) as valsp, \
                     tc.tile_pool(name=f"seg{kk}", bufs=4) as segp, \
                     tc.tile_pool(name=f"hp{kk}", bufs=4,
                                  space="PSUM") as hps:
                    vlo = valsp.tile([P, max(NL, 1) * Dout], F32, tag="vlo",
                                     name="vlo")
                    vhi = valsp.tile([P, max(NH, 1) * Dout], F32, tag="vhi",
                                     name="vhi")
                    for vt, nblk, idx_t, tbl in ((vlo, NL, idxlo_t, tblA),
                                                 (vhi, NH, idxhi_t, tblB)):
                        s0 = 0
                        while s0 < nblk:
                            s1 = min(s0 + SEG_CHUNKS, nblk)
                            nb = s1 - s0
                            nc.gpsimd.dma_gather(
                                out_ap=vt[:, s0 * Dout:s1 * Dout].rearrange(
                                    "p (b f) -> p b f", f=Dout),
                                in_ap=tbl[:, :],
                                idxs_ap=idx_t[:, s0 * 8:s1 * 8],
                                num_idxs=nb * P,
                                num_idxs_reg=nb * P,
                                elem_size=Dout)
                            s0 = s1

                    ofs = [0, 0]
                    for t in range(T):
                        nlo = int(nch_u[t, 0])
                        nhi = int(nch_u[t, 1])
                        chunks = ([(0, ofs[0] + i) for i in range(nlo)]
                                  + [(1, ofs[1] + i) for i in range(nhi)])
                        ofs[0] += nlo
                        ofs[1] += nhi
                        ps = hps.tile([P, Dout], F32)
                        nc.tensor.matmul(
                            out=ps[:, :], lhsT=ident[:],
                            rhs=tloc[kk][:, t * Dout:(t + 1) * Dout],
                            start=True, stop=False)
                        for ci, (h, blk) in enumerate(chunks):
                            vt = vlo if h == 0 else vhi
                            dlt = dllo_t if h == 0 else dlhi_t
                            sg = segp.tile([P, P], F32, name="sg", tag="sg")
                            nc.vector.tensor_tensor(
                                out=sg[:],
                                in0=iota_t[:],
                                in1=dlt[:, blk:blk + 1].broadcast_to(
                                    [P, P]),
                                op=mybir.AluOpType.is_equal)
                            nc.tensor.matmul(
                                out=ps[:, :],
                                lhsT=sg[:],
                                rhs=vt[:, blk * Dout:(blk + 1) * Dout],
                                start=False,
                                stop=(ci == len(chunks) - 1))
                        dst_write(t, ps)

            def hop1_write(t, ps):
                dstv = tloc[1][:, t * Dout:(t + 1) * Dout]
                nc.vector.tensor_tensor(
                    out=dstv, in0=ps[:, :],
                    in1=scl_t[:, T + t:T + t + 1].broadcast_to([P, Dout]),
                    op=mybir.AluOpType.mult)
                if t < TS:
                    nc.sync.dma_start(out=h1sA[t * P:(t + 1) * P, :],
                                      in_=dstv)
                    if t == TS - 1:
                        allgather(h1sA, h1fA)
                else:
                    nc.sync.dma_start(
                        out=h1sB[t * P - RA:(t + 1) * P - RA, :], in_=dstv)

            hop(0, h0fA, h0fB, hop1_write)
            allgather(h1sB, h1fB)

            with tc.tile_pool(name="outp", bufs=3) as outp:

                def out_write(t, ps):
                    ot = outp.tile([P, Dout], F32, name="ot", tag="ot")
                    nc.vector.tensor_tensor(
                        out=ot[:, :], in0=ps[:, :],
                        in1=scl_t[:, 2 * T + t:2 * T + t + 1].broadcast_to(
                            [P, Dout]),
                        op=mybir.AluOpType.mult)
                    nc.sync.dma_start(out=out_d[t * P:(t + 1) * P, :],
                                      in_=ot[:, :])

                hop(1, h1fA, h1fB, out_write)

    nc.compile()
    return nc


def kernel(**inputs):
    global LAST_RESULTS
    x = inputs["x"]
    W = inputs["W"]
    edge_index = inputs["edge_index"]

    dims, nch_u, in_maps, pos = _prepare(x, edge_index, W)
    nc = _build(dims, nch_u)

    trace = bool(int(os.environ.get("GNN_TRACE", "0")))
    kwargs = {}
    if trace:
        kwargs["trace"] = True
        kwargs["trace_cores"] = list(range(NCORES))
    res = run_bass_kernel_spmd(nc, in_maps, core_ids=list(range(NCORES)),
                               **kwargs)
    LAST_RESULTS = res
    full = np.concatenate(
        [res.results[c]["out"] for c in range(NCORES)], axis=0)
    out = full[pos]
    return np.ascontiguousarray(out, dtype=np.float32)



# revision 3
# speedup vs baseline: 1.3630x; 1.3630x over previous
"""SGConv (K=2) GNN message-passing kernel for Trainium2 (8 NeuronCores), v3.

out = (D^{-1/2} (A+I) D^{-1/2})^2 @ x @ W.T

v3 = the proven dma_gather kernel (v1) plus:
  - Host-side balanced node placement: nodes are assigned to
    (core, tile, slot) so that (a) per-core edge counts are equal,
    (b) the A/B table halves carry equal source mass, and (c) every
    (core, tile) needs ~3 A-chunks and ~3 B-chunks -> ~300 gather chunks
    per hop instead of ~420 (28% fewer SWDGE descriptor-gen rows, the
    measured bottleneck).
  - Self-loops removed from the gather stream; each output tile adds its
    local shard tile via an identity matmul (locals stay SBUF-resident).
  - Symmetric-norm folding: table rows are pre-scaled by dinv at
    projection; hop outputs are row-scaled by dinv^2 (hop1) / dinv
    (hop2) on the Activation engine. No per-edge norm multiplies or norm
    tables.
  - Gather batches of 14 chunks (1792 idxs, 113 ring descriptors).

Self-contained: hardcodes NCORES=8; shapes derived from inputs.
"""

import os
import heapq
import numpy as np
import ml_dtypes

from concourse import bacc, mybir, tile
from concourse.bass_utils import run_bass_kernel_spmd

NCORES = 8
P = 128
F32 = mybir.dt.float32
BF16 = mybir.dt.bfloat16
I16 = mybir.dt.int16

SEG_CHUNKS = int(os.environ.get("GNN_SEG", "8"))

LAST_RESULTS = None


def _ceil(a, b):
    return -(-a // b)


def _wrap_idx(idx):
    """int16 [n] -> dma_gather layout [128, n//16]."""
    n = idx.shape[0]
    assert n % 16 == 0
    w = np.ascontiguousarray(idx.reshape(n // 16, 16).T).astype(np.int16)
    return np.ascontiguousarray(np.tile(w, (8, 1)))


def _balance_1d(nodes, weights, nbins, caps):
    """Greedy 1-D: sort by weight desc, assign to lightest bin with space."""
    order = nodes[np.argsort(-weights[nodes], kind="stable")]
    bin_of = np.full(len(weights), -1, np.int32)
    cnt = [0] * nbins
    load = [0.0] * nbins
    heap = [(0.0, b) for b in range(nbins)]
    heapq.heapify(heap)
    for v in order:
        spill = []
        while True:
            w, b = heapq.heappop(heap)
            if cnt[b] < caps[b]:
                break
            spill.append((w, b))
        bin_of[v] = b
        cnt[b] += 1
        load[b] += weights[v]
        heapq.heappush(heap, (load[b], b))
        for it in spill:
            heapq.heappush(heap, it)
    return bin_of


def _balance_2d(nodes, wa, wb, nbins, caps, cap_a, cap_b):
    """Greedy 2-D: sort by total desc; place in the bin (with node space)
    that minimizes the resulting max(A/capA, B/capB). Then a swap-repair
    pass pulls every bin under (cap_a, cap_b) where feasible."""
    order = nodes[np.argsort(-(wa[nodes] + wb[nodes]), kind="stable")]
    bin_of = np.full(len(wa), -1, np.int32)
    cnt = np.zeros(nbins, int)
    la = np.zeros(nbins)
    lb = np.zeros(nbins)
    for v in order:
        cand = np.nonzero(cnt < caps)[0]
        cost = np.maximum((la[cand] + wa[v]) / cap_a,
                          (lb[cand] + wb[v]) / cap_b)
        b = cand[np.argmin(cost)]
        bin_of[v] = b
        cnt[b] += 1
        la[b] += wa[v]
        lb[b] += wb[v]

    members = [list(nodes[bin_of[nodes] == t]) for t in range(nbins)]

    def viol(a, b):
        return np.maximum(0.0, a - cap_a) + np.maximum(0.0, b - cap_b)

    def viol_v(a, b):
        return np.maximum(0.0, a - cap_a) + np.maximum(0.0, b - cap_b)

    for _sweep in range(60):
        improved = False
        vl = viol_v(la, lb)
        for t in np.argsort(-vl):
            t = int(t)
            if vl[t] <= 0:
                break
            best = None
            mu = np.asarray(members[t])
            for s in range(nbins):
                if s == t or not members[s]:
                    continue
                mv = np.asarray(members[s])
                da = wa[mu][:, None] - wa[mv][None, :]
                db = wb[mu][:, None] - wb[mv][None, :]
                nv = (viol_v(la[t] - da, lb[t] - db)
                      + viol_v(la[s] + da, lb[s] + db))
                gain = (vl[t] + vl[s]) - nv
                j = int(np.argmax(gain))
                gj = float(gain.ravel()[j])
                if gj > 1e-9 and (best is None or gj > best[0]):
                    ui, vi = np.unravel_index(j, gain.shape)
                    best = (gj, int(mu[ui]), int(mv[vi]), s)
            if best is None:
                continue
            _, u, v, s = best
            members[t].remove(u)
            members[s].remove(v)
            members[t].append(v)
            members[s].append(u)
            bin_of[u] = s
            bin_of[v] = t
            la[t] += wa[v] - wa[u]
            lb[t] += wb[v] - wb[u]
            la[s] += wa[u] - wa[v]
            lb[s] += wb[u] - wb[v]
            vl = viol_v(la, lb)
            improved = True
        if not improved or viol_v(la, lb).sum() <= 0:
            break
    return bin_of


def _prepare(x, edge_index, W):
    x = np.ascontiguousarray(np.asarray(x, dtype=np.float32))
    W = np.ascontiguousarray(np.asarray(W, dtype=np.float32))
    ei = np.asarray(edge_index).astype(np.int64)

    N, Din = x.shape
    Dout = int(W.shape[0])
    assert N % NCORES == 0
    PN = N // NCORES
    T = _ceil(PN, P)
    PNp = T * P
    KT = Din // P
    TS = T // 2 + 1          # 25 tiles in part A
    RA = TS * P              # 3200
    RB = PNp - RA            # 3072
    assert NCORES * RA < 2**15 and NCORES * RB < 2**15

    indeg = np.bincount(ei[1], minlength=N).astype(np.float64)
    outdeg = np.bincount(ei[0], minlength=N).astype(np.float64)
    dinv = (1.0 / np.sqrt(indeg + 1.0)).astype(np.float32)

    allv = np.arange(N)
    core_of = _balance_1d(allv, indeg, NCORES, [PN] * NCORES)

    # Split each core's nodes into halves A (RA slots) / B, equalizing
    # OUT-degree mass so A/B-sourced edge counts match; preserve IN-degree
    # balance by swapping only equal-indeg pairs.
    halfA = np.zeros(N, bool)
    capsA = [P] * TS
    capsB = [P] * (T - TS)
    capsB[-1] = PN - RA - (T - TS - 1) * P   # last tile short
    nA = sum(capsA)
    for c in range(NCORES):
        nodes = allv[core_of == c]
        so = nodes[np.argsort(-outdeg[nodes], kind="stable")]
        sel = np.zeros(len(so), bool)
        sel[0::2] = True
        diff = int(sel.sum()) - nA
        if diff > 0:
            sel[np.nonzero(sel)[0][-diff:]] = False
        elif diff < 0:
            sel[np.nonzero(~sel)[0][diff:]] = True
        A = list(so[sel])
        Bn = list(so[~sel])

        def _mass_swap(A, Bn, keyd, vald, need):
            """Swap equal-keyd pairs to move ~need of vald mass into A."""
            if abs(need) <= 1:
                return
            byd_A, byd_B = {}, {}
            for i, v in enumerate(A):
                byd_A.setdefault(int(keyd[v]), []).append(i)
            for i, v in enumerate(Bn):
                byd_B.setdefault(int(keyd[v]), []).append(i)
            sgn = 1.0 if need > 0 else -1.0
            pairs = []
            for d in byd_A:
                if d not in byd_B:
                    continue
                ai = sorted(byd_A[d], key=lambda i: sgn * vald[A[i]])
                bi = sorted(byd_B[d], key=lambda i: -sgn * vald[Bn[i]])
                for i, j in zip(ai, bi):
                    pairs.append((vald[Bn[j]] - vald[A[i]], i, j))
            pairs.sort(key=lambda t: -sgn * t[0])
            acc = 0.0
            for gain, i, j in pairs:
                if abs(acc) >= abs(need) or sgn * gain <= 0:
                    break
                A[i], Bn[j] = Bn[j], A[i]
                acc += gain

        # per-bin in-mass parity: A holds TS/T of the in-degree mass
        tgt_in = (indeg[A].sum() + indeg[Bn].sum()) * TS / T
        _mass_swap(A, Bn, outdeg, indeg, tgt_in - indeg[A].sum())
        # out-mass parity via equal-indeg swaps (keeps in-mass fixed)
        _mass_swap(A, Bn, indeg, outdeg,
                   (outdeg[Bn].sum() - outdeg[A].sum()) / 2.0)
        halfA[np.asarray(A, dtype=np.int64)] = True

    # edge source-part weights, then per-core skew fix: the A-half's bins
    # can absorb only TS*384 of wa-mass, so Sum_{v in A}(wa-wb) must be
    # ~0. Swap equal-(indeg,outdeg) pairs across halves (preserves the
    # mass balances; only flips source parts of the pair's out-edges).
    def _wab():
        ewa = halfA[ei[0]]
        wa = np.zeros(N)
        wb = np.zeros(N)
        np.add.at(wa, ei[1][ewa], 1)
        np.add.at(wb, ei[1][~ewa], 1)
        return wa, wb

    # Global pass: each core's incoming source-part skew
    # skew_c = (#A-sourced - #B-sourced edges into core c) must stay small
    # (|skew_c| <= ~120 for per-half feasibility). Flip halves of
    # equal-(indeg,outdeg) node pairs (same owner core) chosen by their
    # out-edge target profiles to minimize ||skewvec||.
    M = np.zeros((N, NCORES), np.float64)
    np.add.at(M, (ei[0], core_of[ei[1]]), 1.0)
    sv = (M[halfA].sum(axis=0) - M[~halfA].sum(axis=0))
    rng = np.random.default_rng(0)
    key2 = (core_of.astype(np.int64) * 100000
            + indeg.astype(np.int64) * 64 + outdeg.astype(np.int64))
    cls = {}
    for v in allv:
        cls.setdefault(int(key2[v]), [[], []])[0 if halfA[v] else 1].append(
            int(v))
    cls_keys = [k for k, ab in cls.items() if ab[0] and ab[1]]
    for _ in range(4000):
        if np.abs(sv).max() <= 48:
            break
        best = None
        for _try in range(64):
            k = cls_keys[int(rng.integers(len(cls_keys)))]
            a_list, b_list = cls[k]
            if not a_list or not b_list:
                continue
            u = a_list[int(rng.integers(len(a_list)))]
            v = b_list[int(rng.integers(len(b_list)))]
            d = 2.0 * (M[v] - M[u])
            gain = float(np.abs(sv).sum() - np.abs(sv + d).sum())
            if gain > 0 and (best is None or gain > best[0]):
                best = (gain, u, v, k, d)
        if best is None:
            continue
        _, u, v, k, d = best
        cls[k][0].remove(u)
        cls[k][1].remove(v)
        cls[k][0].append(v)
        cls[k][1].append(u)
        halfA[u] = False
        halfA[v] = True
        sv = sv + d

    wa, wb = _wab()
    skew = wa - wb
    for c in range(NCORES):
        nodes = allv[core_of == c]
        A = list(nodes[halfA[nodes]])
        Bn = list(nodes[~halfA[nodes]])
        need = -skew[A].sum() / 2.0   # want Sum_A skew -> 0
        if abs(need) > 2:
            key2 = (indeg * 64 + outdeg).astype(np.int64)
            _mass_swap2 = []
            byd_A, byd_B = {}, {}
            for i, v in enumerate(A):
                byd_A.setdefault(int(key2[v]), []).append(i)
            for i, v in enumerate(Bn):
                byd_B.setdefault(int(key2[v]), []).append(i)
            sgn = 1.0 if need > 0 else -1.0
            pairs = []
            for d in byd_A:
                if d not in byd_B:
                    continue
                ai = sorted(byd_A[d], key=lambda i: sgn * skew[A[i]])
                bi = sorted(byd_B[d], key=lambda i: -sgn * skew[Bn[i]])
                for i, j in zip(ai, bi):
                    pairs.append((skew[Bn[j]] - skew[A[i]], i, j))
            pairs.sort(key=lambda t: -sgn * t[0])
            acc = 0.0
            for gain, i, j in pairs:
                if abs(acc) >= abs(need) or sgn * gain <= 0:
                    break
                u, v = A[i], Bn[j]
                A[i], Bn[j] = v, u
                halfA[u] = False
                halfA[v] = True
                acc += gain
    wa, wb = _wab()

    tile_of = np.zeros(N, np.int32)
    slot_of = np.zeros(N, np.int32)
    for c in range(NCORES):
        nodes = allv[core_of == c]
        A = nodes[halfA[nodes]]
        Bn = nodes[~halfA[nodes]]
        tb = _balance_2d(A, wa, wb, TS, np.asarray(capsA), 384.0, 384.0)
        tile_of[A] = tb[A]
        for t in range(TS):
            sel = A[tb[A] == t]
            slot_of[sel] = np.arange(len(sel))
        tb2 = _balance_2d(Bn, wa, wb, T - TS, np.asarray(capsB), 384.0,
                          384.0)
        tile_of[Bn] = TS + tb2[Bn]
        for t in range(T - TS):
            sel = Bn[tb2[Bn] == t]
            slot_of[sel] = np.arange(len(sel))

    # Relabel tiles per core (within each half): bins that overflow the
    # A-chunk budget go to the FRONT, B-overflow bins to the BACK, so the
    # cross-core max pays each overflow category once (aligned) instead of
    # once per scattered tile.
    for c in range(NCORES):
        nodes = allv[core_of == c]
        for lo, hi in ((0, TS), (TS, T)):
            keys = []
            for t in range(lo, hi):
                m = nodes[tile_of[nodes] == t]
                ao = _ceil(int(wa[m].sum()), P) > 3
                bo = _ceil(int(wb[m].sum()), P) > 3
                # A-over (incl both-over) front desc; B-over back;
                # normals in the middle
                if ao:
                    k = -2000 - int(wa[m].sum())
                elif bo:
                    k = 2000 + int(wb[m].sum())
                else:
                    k = -int(wa[m].sum() + wb[m].sum())
                keys.append(k)
            order2 = sorted(range(hi - lo), key=lambda i: keys[i])
            remap = np.zeros(hi - lo, np.int32)
            for newt, oldt in enumerate(order2):
                remap[oldt] = newt
            sel = nodes[(tile_of[nodes] >= lo) & (tile_of[nodes] < hi)]
            tile_of[sel] = lo + remap[tile_of[sel] - lo]

    pos = (core_of.astype(np.int64) * PNp + tile_of * P + slot_of)

    # ---- edge lists (no self-loops) ----
    dst_core = core_of[ei[1]]
    dst_tile = tile_of[ei[1]]
    dstloc = slot_of[ei[1]].astype(np.float32)
    s_pos = pos[ei[0]]
    s_core = s_pos // PNp
    s_off = s_pos % PNp
    part = (s_off >= RA).astype(np.int64)
    srcloc = np.where(part == 1,
                      s_core * RB + (s_off - RA),
                      s_core * RA + s_off)

    key = (dst_core * T + dst_tile) * 2 + part
    order = np.argsort(key, kind="stable")
    s_srcloc = srcloc[order]
    s_dstloc = dstloc[order]

    cnt = np.zeros(NCORES * T * 2, np.int64)
    np.add.at(cnt, key, 1)
    cnt = cnt.reshape(NCORES, T, 2)
    nch = _ceil(cnt, P)
    nch_u = nch.max(axis=0)            # [T, 2]
    NL = int(nch_u[:, 0].sum())
    NH = int(nch_u[:, 1].sum())

    starts = np.zeros(NCORES * T * 2 + 1, np.int64)
    starts[1:] = np.cumsum(cnt.reshape(-1))

    iota = np.ascontiguousarray(
        np.tile(np.arange(P, dtype=np.float32), (P, 1)))
    ident = np.ascontiguousarray(np.eye(P, dtype=np.float32))
    wt = np.ascontiguousarray(
        W.T.reshape(KT, P, Dout).transpose(1, 0, 2).reshape(P, KT * Dout))

    dinv_p = np.zeros(NCORES * PNp, np.float32)
    dinv_p[pos] = dinv

    in_maps = []
    for c in range(NCORES):
        idx_f = [np.zeros(NL * P, np.int64), np.zeros(NH * P, np.int64)]
        dl_f = [np.full(NL * P, -1.0, np.float32),
                np.full(NH * P, -1.0, np.float32)]
        off = [0, 0]
        for t in range(T):
            for h in (0, 1):
                k = (c * T + t) * 2 + h
                a, b = int(starts[k]), int(starts[k + 1])
                n = b - a
                o = off[h] * P
                idx_f[h][o:o + n] = s_srcloc[a:b]
                dl_f[h][o:o + n] = s_dstloc[a:b]
                off[h] += int(nch_u[t, h])
        assert off[0] == NL and off[1] == NH

        mine = np.nonzero(core_of == c)[0]
        xp = np.zeros((PNp, Din), np.float32)
        xp[tile_of[mine] * P + slot_of[mine]] = x[mine]
        xt = np.ascontiguousarray(
            xp.T.reshape(KT, P, PNp).transpose(1, 0, 2).reshape(
                P, KT * PNp).astype(ml_dtypes.bfloat16))

        dv = np.ascontiguousarray(
            dinv_p[c * PNp:(c + 1) * PNp].reshape(T, P).T)  # [P, T]
        scl = np.ascontiguousarray(
            np.concatenate([dv, dv * dv, dv], axis=1))      # [P, 3T]

        in_maps.append({
            "xt": xt,
            "wt": np.ascontiguousarray(wt.astype(ml_dtypes.bfloat16)),
            "iota": iota,
            "ident": ident,
            "scl": scl,
            "idxlo": _wrap_idx(idx_f[0].astype(np.int16)),
            "idxhi": _wrap_idx(idx_f[1].astype(np.int16)),
            "dllo": np.ascontiguousarray(dl_f[0].reshape(NL, P).T),
            "dlhi": np.ascontiguousarray(dl_f[1].reshape(NH, P).T),
        })

    dims = dict(N=N, PNp=PNp, T=T, KT=KT, Din=Din, Dout=Dout, TS=TS,
                RA=RA, RB=RB)
    return dims, nch_u, in_maps, pos


def _build(dims, nch_u):
    PNp, T, KT, Dout = dims["PNp"], dims["T"], dims["KT"], dims["Dout"]
    TS, RA, RB = dims["TS"], dims["RA"], dims["RB"]
    NL = int(nch_u[:, 0].sum())
    NH = int(nch_u[:, 1].sum())

    nc = bacc.Bacc("TRN2", target_bir_lowering=False, debug=False,
                   num_devices=NCORES, num_swdge_queues=4)

    xt_d = nc.dram_tensor("xt", [P, KT * PNp], BF16,
                      kind="ExternalInput")
    wt_d = nc.dram_tensor("wt", [P, KT * Dout], BF16,
                          kind="ExternalInput")
    iota_d = nc.dram_tensor("iota", [P, P], F32, kind="ExternalInput")
    id_d = nc.dram_tensor("ident", [P, P], F32, kind="ExternalInput")
    scl_d = nc.dram_tensor("scl", [P, 3 * T], F32, kind="ExternalInput")
    idxlo_d = nc.dram_tensor("idxlo", [P, NL * 8], I16, kind="ExternalInput")
    idxhi_d = nc.dram_tensor("idxhi", [P, NH * 8], I16, kind="ExternalInput")
    dllo_d = nc.dram_tensor("dllo", [P, NL], F32, kind="ExternalInput")
    dlhi_d = nc.dram_tensor("dlhi", [P, NH], F32, kind="ExternalInput")
    out_d = nc.dram_tensor("out", [PNp, Dout], F32, kind="ExternalOutput")

    h0sA = nc.dram_tensor("h0sA", [RA, Dout], F32)
    h0sB = nc.dram_tensor("h0sB", [RB, Dout], F32)
    h0fA = nc.dram_tensor("h0fA", [NCORES * RA, Dout], F32,
                          addr_space="Shared")
    h0fB = nc.dram_tensor("h0fB", [NCORES * RB, Dout], F32,
                          addr_space="Shared")
    h1sA = nc.dram_tensor("h1sA", [RA, Dout], F32)
    h1sB = nc.dram_tensor("h1sB", [RB, Dout], F32)
    h1fA = nc.dram_tensor("h1fA", [NCORES * RA, Dout], F32,
                          addr_space="Shared")
    h1fB = nc.dram_tensor("h1fB", [NCORES * RB, Dout], F32,
                          addr_space="Shared")

    rg = [list(range(NCORES))]

    def allgather(src, dst):
        nc.gpsimd.collective_compute(
            "AllGather", mybir.AluOpType.bypass, replica_groups=rg,
            ins=[src.ap().opt()], outs=[dst.ap().opt()])

    with tile.TileContext(nc) as tc:
        with tc.tile_pool(name="const", bufs=1) as constp:
            wts = constp.tile([P, KT * Dout], BF16)
            nc.sync.dma_start(out=wts[:], in_=wt_d[:, :])
            iota_t = constp.tile([P, P], F32)
            nc.sync.dma_start(out=iota_t[:], in_=iota_d[:, :])
            ident = constp.tile([P, P], F32)
            nc.sync.dma_start(out=ident[:], in_=id_d[:, :])
            scl_t = constp.tile([P, 3 * T], F32)
            nc.sync.dma_start(out=scl_t[:], in_=scl_d[:, :])
            idxlo_t = constp.tile([P, NL * 8], I16)
            nc.sync.dma_start(out=idxlo_t[:], in_=idxlo_d[:, :])
            idxhi_t = constp.tile([P, NH * 8], I16)
            nc.sync.dma_start(out=idxhi_t[:], in_=idxhi_d[:, :])
            dllo_t = constp.tile([P, NL], F32)
            nc.sync.dma_start(out=dllo_t[:], in_=dllo_d[:, :])
            dlhi_t = constp.tile([P, NH], F32)
            nc.sync.dma_start(out=dlhi_t[:], in_=dlhi_d[:, :])

            # hop-input locals (row-major, dinv-scaled), SBUF-resident
            tloc = [constp.tile([P, T * Dout], F32, tag=f"tloc{k}",
                                name=f"tloc{k}")
                    for k in range(2)]

            # ---------------- projection: t0 = dinv * (x @ W.T) ----------
            with tc.tile_pool(name="proj", bufs=1) as projp, \
                 tc.tile_pool(name="ppsum", bufs=4, space="PSUM") as ppsum:
                xts = projp.tile([P, KT * PNp], BF16)
                for k in range(KT):
                    for lo, ln in ((0, RA), (RA, RB)):
                        nc.sync.dma_start(
                            out=xts[:, k * PNp + lo:k * PNp + lo + ln],
                            in_=xt_d[:, k * PNp + lo:k * PNp + lo + ln])
                for m in range(T):
                    ps = ppsum.tile([P, Dout], F32)
                    for k in range(KT):
                        nc.tensor.matmul(
                            out=ps[:, :],
                            lhsT=xts[:,
                                     k * PNp + m * P:k * PNp + (m + 1) * P],
                            rhs=wts[:, k * Dout:(k + 1) * Dout],
                            start=(k == 0), stop=(k == KT - 1))
                    dstv = tloc[0][:, m * Dout:(m + 1) * Dout]
                    nc.vector.tensor_tensor(
                        out=dstv, in0=ps[:, :],
                        in1=scl_t[:, m:m + 1].broadcast_to([P, Dout]),
                        op=mybir.AluOpType.mult)
                    if m < TS:
                        nc.sync.dma_start(
                            out=h0sA[m * P:(m + 1) * P, :], in_=dstv)
                    else:
                        nc.sync.dma_start(
                            out=h0sB[m * P - RA:(m + 1) * P - RA, :],
                            in_=dstv)
                    if m == TS - 1:
                        allgather(h0sA, h0fA)
                allgather(h0sB, h0fB)

            def hop(kk, tblA, tblB, dst_write):
                with tc.tile_pool(name=f"vals{kk}", bufs=1) as valsp, \
                     tc.tile_pool(name=f"seg{kk}", bufs=4) as segp, \
                     tc.tile_pool(name=f"hp{kk}", bufs=4,
                                  space="PSUM") as hps:
                    vlo = valsp.tile([P, max(NL, 1) * Dout], F32, tag="vlo",
                                     name="vlo")
                    vhi = valsp.tile([P, max(NH, 1) * Dout], F32, tag="vhi",
                                     name="vhi")
                    qn = 0
                    for vt, nblk, idx_t, tbl in ((vlo, NL, idxlo_t, tblA),
                                                 (vhi, NH, idxhi_t, tblB)):
                        s0 = 0
                        while s0 < nblk:
                            s1 = min(s0 + SEG_CHUNKS, nblk)
                            nb = s1 - s0
                            nc.gpsimd.dma_gather(
                                out_ap=vt[:, s0 * Dout:s1 * Dout].rearrange(
                                    "p (b f) -> p b f", f=Dout),
                                in_ap=tbl[:, :],
                                idxs_ap=idx_t[:, s0 * 8:s1 * 8],
                                num_idxs=nb * P,
                                num_idxs_reg=nb * P,
                                elem_size=Dout,
                                queue_num=qn % 4)
                            qn += 1
                            s0 = s1

                    ofs = [0, 0]
                    for t in range(T):
                        nlo = int(nch_u[t, 0])
                        nhi = int(nch_u[t, 1])
                        chunks = ([(0, ofs[0] + i) for i in range(nlo)]
                                  + [(1, ofs[1] + i) for i in range(nhi)])
                        ofs[0] += nlo
                        ofs[1] += nhi
                        ps = hps.tile([P, Dout], F32)
                        nc.tensor.matmul(
                            out=ps[:, :], lhsT=ident[:],
                            rhs=tloc[kk][:, t * Dout:(t + 1) * Dout],
                            start=True, stop=False)
                        for ci, (h, blk) in enumerate(chunks):
                            vt = vlo if h == 0 else vhi
                            dlt = dllo_t if h == 0 else dlhi_t
                            sg = segp.tile([P, P], F32, name="sg", tag="sg")
                            nc.vector.tensor_tensor(
                                out=sg[:],
                                in0=iota_t[:],
                                in1=dlt[:, blk:blk + 1].broadcast_to(
                                    [P, P]),
                                op=mybir.AluOpType.is_equal)
                            nc.tensor.matmul(
                                out=ps[:, :],
                                lhsT=sg[:],
                                rhs=vt[:, blk * Dout:(blk + 1) * Dout],
                                start=False,
                                stop=(ci == len(chunks) - 1))
                        dst_write(t, ps)

            def hop1_write(t, ps):
                dstv = tloc[1][:, t * Dout:(t + 1) * Dout]
                nc.vector.tensor_tensor(
                    out=dstv, in0=ps[:, :],
                    in1=scl_t[:, T + t:T + t + 1].broadcast_to([P, Dout]),
                    op=mybir.AluOpType.mult)
                if t < TS:
                    nc.sync.dma_start(out=h1sA[t * P:(t + 1) * P, :],
                                      in_=dstv)
                    if t == TS - 1:
                        allgather(h1sA, h1fA)
                else:
                    nc.sync.dma_start(
                        out=h1sB[t * P - RA:(t + 1) * P - RA, :], in_=dstv)

            hop(0, h0fA, h0fB, hop1_write)
            allgather(h1sB, h1fB)

            with tc.tile_pool(name="outp", bufs=3) as outp:

                def out_write(t, ps):
                    ot = outp.tile([P, Dout], F32, name="ot", tag="ot")
                    nc.vector.tensor_tensor(
                        out=ot[:, :], in0=ps[:, :],
                        in1=scl_t[:, 2 * T + t:2 * T + t + 1].broadcast_to(
                            [P, Dout]),
                        op=mybir.AluOpType.mult)
                    nc.sync.dma_start(out=out_d[t * P:(t + 1) * P, :],
                                      in_=ot[:, :])

                hop(1, h1fA, h1fB, out_write)

    nc.compile()
    return nc


def kernel(**inputs):
    global LAST_RESULTS
    x = inputs["x"]
    W = inputs["W"]
    edge_index = inputs["edge_index"]

    dims, nch_u, in_maps, pos = _prepare(x, edge_index, W)
    nc = _build(dims, nch_u)

    trace = bool(int(os.environ.get("GNN_TRACE", "0")))
    kwargs = {}
    if trace:
        kwargs["trace"] = True
        kwargs["trace_cores"] = list(range(NCORES))
    res = run_bass_kernel_spmd(nc, in_maps, core_ids=list(range(NCORES)),
                               **kwargs)
    LAST_RESULTS = res
    full = np.concatenate(
        [res.results[c]["out"] for c in range(NCORES)], axis=0)
    out = full[pos]
    return np.ascontiguousarray(out, dtype=np.float32)



# revision 13
# speedup vs baseline: 2.0208x; 1.4826x over previous
"""SGConv (K=2) GNN message-passing kernel for Trainium2 (8 NeuronCores), v3.

out = (D^{-1/2} (A+I) D^{-1/2})^2 @ x @ W.T

v3 = the proven dma_gather kernel (v1) plus:
  - Host-side balanced node placement: nodes are assigned to
    (core, tile, slot) so that (a) per-core edge counts are equal,
    (b) the A/B table halves carry equal source mass, and (c) every
    (core, tile) needs ~3 A-chunks and ~3 B-chunks -> ~300 gather chunks
    per hop instead of ~420 (28% fewer SWDGE descriptor-gen rows, the
    measured bottleneck).
  - Self-loops removed from the gather stream; each output tile adds its
    local shard tile via an identity matmul (locals stay SBUF-resident).
  - Symmetric-norm folding: table rows are pre-scaled by dinv at
    projection; hop outputs are row-scaled by dinv^2 (hop1) / dinv
    (hop2) on the Activation engine. No per-edge norm multiplies or norm
    tables.
  - Gather batches of 14 chunks (1792 idxs, 113 ring descriptors).

Self-contained: hardcodes NCORES=8; shapes derived from inputs.
"""

import os
import heapq
import numpy as np
import ml_dtypes

from concourse import bacc, mybir, tile
from concourse.bass_utils import run_bass_kernel_spmd

NCORES = 8
P = 128
F32 = mybir.dt.float32
BF16 = mybir.dt.bfloat16
I16 = mybir.dt.int16

SEG_CHUNKS = int(os.environ.get("GNN_SEG", "8"))

LAST_RESULTS = None


def _ceil(a, b):
    return -(-a // b)


def _wrap_idx(idx):
    """int16 [n] -> dma_gather layout [128, n//16]."""
    n = idx.shape[0]
    assert n % 16 == 0
    w = np.ascontiguousarray(idx.reshape(n // 16, 16).T).astype(np.int16)
    return np.ascontiguousarray(np.tile(w, (8, 1)))


def _balance_1d(nodes, weights, nbins, caps):
    """Greedy 1-D: sort by weight desc, assign to lightest bin with space."""
    order = nodes[np.argsort(-weights[nodes], kind="stable")]
    bin_of = np.full(len(weights), -1, np.int32)
    cnt = [0] * nbins
    load = [0.0] * nbins
    heap = [(0.0, b) for b in range(nbins)]
    heapq.heapify(heap)
    for v in order:
        spill = []
        while True:
            w, b = heapq.heappop(heap)
            if cnt[b] < caps[b]:
                break
            spill.append((w, b))
        bin_of[v] = b
        cnt[b] += 1
        load[b] += weights[v]
        heapq.heappush(heap, (load[b], b))
        for it in spill:
            heapq.heappush(heap, it)
    return bin_of


def _balance_2d(nodes, wa, wb, nbins, caps, cap_a, cap_b):
    """Greedy 2-D: sort by total desc; place in the bin (with node space)
    that minimizes the resulting max(A/capA, B/capB). Then a swap-repair
    pass pulls every bin under (cap_a, cap_b) where feasible."""
    order = nodes[np.argsort(-(wa[nodes] + wb[nodes]), kind="stable")]
    bin_of = np.full(len(wa), -1, np.int32)
    cnt = np.zeros(nbins, int)
    la = np.zeros(nbins)
    lb = np.zeros(nbins)
    for v in order:
        cand = np.nonzero(cnt < caps)[0]
        cost = np.maximum((la[cand] + wa[v]) / cap_a,
                          (lb[cand] + wb[v]) / cap_b)
        b = cand[np.argmin(cost)]
        bin_of[v] = b
        cnt[b] += 1
        la[b] += wa[v]
        lb[b] += wb[v]

    members = [list(nodes[bin_of[nodes] == t]) for t in range(nbins)]

    def viol(a, b):
        return np.maximum(0.0, a - cap_a) + np.maximum(0.0, b - cap_b)

    def viol_v(a, b):
        return np.maximum(0.0, a - cap_a) + np.maximum(0.0, b - cap_b)

    for _sweep in range(60):
        improved = False
        vl = viol_v(la, lb)
        for t in np.argsort(-vl):
            t = int(t)
            if vl[t] <= 0:
                break
            best = None
            mu = np.asarray(members[t])
            for s in range(nbins):
                if s == t or not members[s]:
                    continue
                mv = np.asarray(members[s])
                da = wa[mu][:, None] - wa[mv][None, :]
                db = wb[mu][:, None] - wb[mv][None, :]
                nv = (viol_v(la[t] - da, lb[t] - db)
                      + viol_v(la[s] + da, lb[s] + db))
                gain = (vl[t] + vl[s]) - nv
                j = int(np.argmax(gain))
                gj = float(gain.ravel()[j])
                if gj > 1e-9 and (best is None or gj > best[0]):
                    ui, vi = np.unravel_index(j, gain.shape)
                    best = (gj, int(mu[ui]), int(mv[vi]), s)
            if best is None:
                continue
            _, u, v, s = best
            members[t].remove(u)
            members[s].remove(v)
            members[t].append(v)
            members[s].append(u)
            bin_of[u] = s
            bin_of[v] = t
            la[t] += wa[v] - wa[u]
            lb[t] += wb[v] - wb[u]
            la[s] += wa[u] - wa[v]
            lb[s] += wb[u] - wb[v]
            vl = viol_v(la, lb)
            improved = True
        if not improved or viol_v(la, lb).sum() <= 0:
            break
    return bin_of


def _prepare(x, edge_index, W):
    x = np.ascontiguousarray(np.asarray(x, dtype=np.float32))
    W = np.ascontiguousarray(np.asarray(W, dtype=np.float32))
    ei = np.asarray(edge_index).astype(np.int64)

    N, Din = x.shape
    Dout = int(W.shape[0])
    assert N % NCORES == 0
    PN = N // NCORES
    T = _ceil(PN, P)
    PNp = T * P
    KT = Din // P
    TS = T // 2 + 1          # 25 tiles in part A
    RA = TS * P              # 3200
    RB = PNp - RA            # 3072
    assert NCORES * RA < 2**15 and NCORES * RB < 2**15

    indeg = np.bincount(ei[1], minlength=N).astype(np.float64)
    outdeg = np.bincount(ei[0], minlength=N).astype(np.float64)
    dinv = (1.0 / np.sqrt(indeg + 1.0)).astype(np.float32)

    allv = np.arange(N)
    core_of = _balance_1d(allv, indeg, NCORES, [PN] * NCORES)

    # Split each core's nodes into halves A (RA slots) / B, equalizing
    # OUT-degree mass so A/B-sourced edge counts match; preserve IN-degree
    # balance by swapping only equal-indeg pairs.
    halfA = np.zeros(N, bool)
    capsA = [P] * TS
    capsB = [P] * (T - TS)
    capsB[-1] = PN - RA - (T - TS - 1) * P   # last tile short
    nA = sum(capsA)
    for c in range(NCORES):
        nodes = allv[core_of == c]
        so = nodes[np.argsort(-outdeg[nodes], kind="stable")]
        sel = np.zeros(len(so), bool)
        sel[0::2] = True
        diff = int(sel.sum()) - nA
        if diff > 0:
            sel[np.nonzero(sel)[0][-diff:]] = False
        elif diff < 0:
            sel[np.nonzero(~sel)[0][diff:]] = True
        A = list(so[sel])
        Bn = list(so[~sel])

        def _mass_swap(A, Bn, keyd, vald, need):
            """Swap equal-keyd pairs to move ~need of vald mass into A."""
            if abs(need) <= 1:
                return
            byd_A, byd_B = {}, {}
            for i, v in enumerate(A):
                byd_A.setdefault(int(keyd[v]), []).append(i)
            for i, v in enumerate(Bn):
                byd_B.setdefault(int(keyd[v]), []).append(i)
            sgn = 1.0 if need > 0 else -1.0
            pairs = []
            for d in byd_A:
                if d not in byd_B:
                    continue
                ai = sorted(byd_A[d], key=lambda i: sgn * vald[A[i]])
                bi = sorted(byd_B[d], key=lambda i: -sgn * vald[Bn[i]])
                for i, j in zip(ai, bi):
                    pairs.append((vald[Bn[j]] - vald[A[i]], i, j))
            pairs.sort(key=lambda t: -sgn * t[0])
            acc = 0.0
            for gain, i, j in pairs:
                if abs(acc) >= abs(need) or sgn * gain <= 0:
                    break
                A[i], Bn[j] = Bn[j], A[i]
                acc += gain

        # per-bin in-mass parity: A holds TS/T of the in-degree mass
        tgt_in = (indeg[A].sum() + indeg[Bn].sum()) * TS / T
        _mass_swap(A, Bn, outdeg, indeg, tgt_in - indeg[A].sum())
        # out-mass parity via equal-indeg swaps (keeps in-mass fixed)
        _mass_swap(A, Bn, indeg, outdeg,
                   (outdeg[Bn].sum() - outdeg[A].sum()) / 2.0)
        halfA[np.asarray(A, dtype=np.int64)] = True

    # edge source-part weights, then per-core skew fix: the A-half's bins
    # can absorb only TS*384 of wa-mass, so Sum_{v in A}(wa-wb) must be
    # ~0. Swap equal-(indeg,outdeg) pairs across halves (preserves the
    # mass balances; only flips source parts of the pair's out-edges).
    def _wab():
        ewa = halfA[ei[0]]
        wa = np.zeros(N)
        wb = np.zeros(N)
        np.add.at(wa, ei[1][ewa], 1)
        np.add.at(wb, ei[1][~ewa], 1)
        return wa, wb

    # Global pass: each core's incoming source-part skew
    # skew_c = (#A-sourced - #B-sourced edges into core c) must stay small
    # (|skew_c| <= ~120 for per-half feasibility). Flip halves of
    # equal-(indeg,outdeg) node pairs (same owner core) chosen by their
    # out-edge target profiles to minimize ||skewvec||.
    M = np.zeros((N, NCORES), np.float64)
    np.add.at(M, (ei[0], core_of[ei[1]]), 1.0)
    sv = (M[halfA].sum(axis=0) - M[~halfA].sum(axis=0))
    rng = np.random.default_rng(0)
    key2 = (core_of.astype(np.int64) * 100000
            + indeg.astype(np.int64) * 64 + outdeg.astype(np.int64))
    cls = {}
    for v in allv:
        cls.setdefault(int(key2[v]), [[], []])[0 if halfA[v] else 1].append(
            int(v))
    cls_keys = [k for k, ab in cls.items() if ab[0] and ab[1]]
    for _ in range(4000):
        if np.abs(sv).max() <= 48:
            break
        best = None
        for _try in range(64):
            k = cls_keys[int(rng.integers(len(cls_keys)))]
            a_list, b_list = cls[k]
            if not a_list or not b_list:
                continue
            u = a_list[int(rng.integers(len(a_list)))]
            v = b_list[int(rng.integers(len(b_list)))]
            d = 2.0 * (M[v] - M[u])
            gain = float(np.abs(sv).sum() - np.abs(sv + d).sum())
            if gain > 0 and (best is None or gain > best[0]):
                best = (gain, u, v, k, d)
        if best is None:
            continue
        _, u, v, k, d = best
        cls[k][0].remove(u)
        cls[k][1].remove(v)
        cls[k][0].append(v)
        cls[k][1].append(u)
        halfA[u] = False
        halfA[v] = True
        sv = sv + d

    wa, wb = _wab()
    skew = wa - wb
    for c in range(NCORES):
        nodes = allv[core_of == c]
        A = list(nodes[halfA[nodes]])
        Bn = list(nodes[~halfA[nodes]])
        need = -skew[A].sum() / 2.0   # want Sum_A skew -> 0
        if abs(need) > 2:
            key2 = (indeg * 64 + outdeg).astype(np.int64)
            _mass_swap2 = []
            byd_A, byd_B = {}, {}
            for i, v in enumerate(A):
                byd_A.setdefault(int(key2[v]), []).append(i)
            for i, v in enumerate(Bn):
                byd_B.setdefault(int(key2[v]), []).append(i)
            sgn = 1.0 if need > 0 else -1.0
            pairs = []
            for d in byd_A:
                if d not in byd_B:
                    continue
                ai = sorted(byd_A[d], key=lambda i: sgn * skew[A[i]])
                bi = sorted(byd_B[d], key=lambda i: -sgn * skew[Bn[i]])
                for i, j in zip(ai, bi):
                    pairs.append((skew[Bn[j]] - skew[A[i]], i, j))
            pairs.sort(key=lambda t: -sgn * t[0])
            acc = 0.0
            for gain, i, j in pairs:
                if abs(acc) >= abs(need) or sgn * gain <= 0:
                    break
                u, v = A[i], Bn[j]
                A[i], Bn[j] = v, u
                halfA[u] = False
                halfA[v] = True
                acc += gain
    wa, wb = _wab()

    tile_of = np.zeros(N, np.int32)
    slot_of = np.zeros(N, np.int32)
    for c in range(NCORES):
        nodes = allv[core_of == c]
        A = nodes[halfA[nodes]]
        Bn = nodes[~halfA[nodes]]
        tb = _balance_2d(A, wa, wb, TS, np.asarray(capsA), 384.0, 384.0)
        tile_of[A] = tb[A]
        for t in range(TS):
            sel = A[tb[A] == t]
            slot_of[sel] = np.arange(len(sel))
        tb2 = _balance_2d(Bn, wa, wb, T - TS, np.asarray(capsB), 384.0,
                          384.0)
        tile_of[Bn] = TS + tb2[Bn]
        for t in range(T - TS):
            sel = Bn[tb2[Bn] == t]
            slot_of[sel] = np.arange(len(sel))

    # Relabel tiles per core (within each half): bins that overflow the
    # A-chunk budget go to the FRONT, B-overflow bins to the BACK, so the
    # cross-core max pays each overflow category once (aligned) instead of
    # once per scattered tile.
    for c in range(NCORES):
        nodes = allv[core_of == c]
        for lo, hi in ((0, TS), (TS, T)):
            keys = []
            for t in range(lo, hi):
                m = nodes[tile_of[nodes] == t]
                ao = _ceil(int(wa[m].sum()), P) > 3
                bo = _ceil(int(wb[m].sum()), P) > 3
                # A-over (incl both-over) front desc; B-over back;
                # normals in the middle
                if ao:
                    k = -2000 - int(wa[m].sum())
                elif bo:
                    k = 2000 + int(wb[m].sum())
                else:
                    k = -int(wa[m].sum() + wb[m].sum())
                keys.append(k)
            order2 = sorted(range(hi - lo), key=lambda i: keys[i])
            remap = np.zeros(hi - lo, np.int32)
            for newt, oldt in enumerate(order2):
                remap[oldt] = newt
            sel = nodes[(tile_of[nodes] >= lo) & (tile_of[nodes] < hi)]
            tile_of[sel] = lo + remap[tile_of[sel] - lo]

    pos = (core_of.astype(np.int64) * PNp + tile_of * P + slot_of)

    # ---- edge lists (no self-loops) ----
    dst_core = core_of[ei[1]]
    dst_tile = tile_of[ei[1]]
    dstloc = slot_of[ei[1]].astype(np.float32)
    s_pos = pos[ei[0]]
    s_core = s_pos // PNp
    s_off = s_pos % PNp
    part = (s_off >= RA).astype(np.int64)
    srcloc = np.where(part == 1,
                      s_core * RB + (s_off - RA),
                      s_core * RA + s_off)

    key = (dst_core * T + dst_tile) * 2 + part
    order = np.argsort(key, kind="stable")
    s_srcloc = srcloc[order]
    s_dstloc = dstloc[order]

    cnt = np.zeros(NCORES * T * 2, np.int64)
    np.add.at(cnt, key, 1)
    cnt = cnt.reshape(NCORES, T, 2)
    nch = _ceil(cnt, P)
    nch_u = nch.max(axis=0)            # [T, 2]
    NL = int(nch_u[:, 0].sum())
    NH = int(nch_u[:, 1].sum())

    starts = np.zeros(NCORES * T * 2 + 1, np.int64)
    starts[1:] = np.cumsum(cnt.reshape(-1))

    iota = np.ascontiguousarray(
        np.tile(np.arange(P, dtype=np.float32), (P, 1)))
    ident = np.ascontiguousarray(np.eye(P, dtype=ml_dtypes.bfloat16))
    wt = np.ascontiguousarray(
        W.T.reshape(KT, P, Dout).transpose(1, 0, 2).reshape(P, KT * Dout))

    dinv_p = np.zeros(NCORES * PNp, np.float32)
    dinv_p[pos] = dinv

    in_maps = []
    for c in range(NCORES):
        idx_f = [np.zeros(NL * P, np.int64), np.zeros(NH * P, np.int64)]
        dl_f = [np.full(NL * P, -1.0, np.float32),
                np.full(NH * P, -1.0, np.float32)]
        off = [0, 0]
        for t in range(T):
            for h in (0, 1):
                k = (c * T + t) * 2 + h
                a, b = int(starts[k]), int(starts[k + 1])
                n = b - a
                o = off[h] * P
                idx_f[h][o:o + n] = s_srcloc[a:b]
                dl_f[h][o:o + n] = s_dstloc[a:b]
                off[h] += int(nch_u[t, h])
        assert off[0] == NL and off[1] == NH

        mine = np.nonzero(core_of == c)[0]
        xp = np.zeros((PNp, Din), np.float32)
        xp[tile_of[mine] * P + slot_of[mine]] = x[mine]
        xt = np.ascontiguousarray(
            xp.T.reshape(KT, P, PNp).transpose(1, 0, 2).reshape(
                P, KT * PNp).astype(ml_dtypes.bfloat16))

        dv = np.ascontiguousarray(
            dinv_p[c * PNp:(c + 1) * PNp].reshape(T, P).T)  # [P, T]
        scl = np.ascontiguousarray(
            np.concatenate([dv, dv * dv, dv], axis=1))      # [P, 3T]

        in_maps.append({
            "xt": xt,
            "wt": np.ascontiguousarray(wt.astype(ml_dtypes.bfloat16)),
            "iota": iota,
            "ident": ident,
            "scl": scl,
            "idxlo": _wrap_idx(idx_f[0].astype(np.int16)),
            "idxhi": _wrap_idx(idx_f[1].astype(np.int16)),
            "dllo": np.ascontiguousarray(dl_f[0].reshape(NL, P).T),
            "dlhi": np.ascontiguousarray(dl_f[1].reshape(NH, P).T),
        })

    dims = dict(N=N, PNp=PNp, T=T, KT=KT, Din=Din, Dout=Dout, TS=TS,
                RA=RA, RB=RB)
    return dims, nch_u, in_maps, pos


def _build(dims, nch_u):
    PNp, T, KT, Dout = dims["PNp"], dims["T"], dims["KT"], dims["Dout"]
    TS, RA, RB = dims["TS"], dims["RA"], dims["RB"]
    NL = int(nch_u[:, 0].sum())
    NH = int(nch_u[:, 1].sum())

    nc = bacc.Bacc("TRN2", target_bir_lowering=False, debug=False,
                   num_devices=NCORES, num_swdge_queues=4)

    xt_d = nc.dram_tensor("xt", [P, KT * PNp], BF16,
                      kind="ExternalInput")
    wt_d = nc.dram_tensor("wt", [P, KT * Dout], BF16,
                          kind="ExternalInput")
    iota_d = nc.dram_tensor("iota", [P, P], F32, kind="ExternalInput")
    id_d = nc.dram_tensor("ident", [P, P], BF16, kind="ExternalInput")
    scl_d = nc.dram_tensor("scl", [P, 3 * T], F32, kind="ExternalInput")
    idxlo_d = nc.dram_tensor("idxlo", [P, NL * 8], I16, kind="ExternalInput")
    idxhi_d = nc.dram_tensor("idxhi", [P, NH * 8], I16, kind="ExternalInput")
    dllo_d = nc.dram_tensor("dllo", [P, NL], F32, kind="ExternalInput")
    dlhi_d = nc.dram_tensor("dlhi", [P, NH], F32, kind="ExternalInput")
    out_d = nc.dram_tensor("out", [PNp, Dout], F32, kind="ExternalOutput")

    DR = 2 * Dout   # 128 bf16 = 256B rows (only first Dout cols are real)
    h0sA = nc.dram_tensor("h0sA", [RA, DR], BF16)
    h0sB = nc.dram_tensor("h0sB", [RB, DR], BF16)
    h0fA = nc.dram_tensor("h0fA", [NCORES * RA, DR], BF16,
                          addr_space="Shared")
    h0fB = nc.dram_tensor("h0fB", [NCORES * RB, DR], BF16,
                          addr_space="Shared")
    h1sA = nc.dram_tensor("h1sA", [RA, DR], BF16)
    h1sB = nc.dram_tensor("h1sB", [RB, DR], BF16)
    h1fA = nc.dram_tensor("h1fA", [NCORES * RA, DR], BF16,
                          addr_space="Shared")
    h1fB = nc.dram_tensor("h1fB", [NCORES * RB, DR], BF16,
                          addr_space="Shared")

    rg = [list(range(NCORES))]

    def allgather(src, dst):
        nc.gpsimd.collective_compute(
            "AllGather", mybir.AluOpType.bypass, replica_groups=rg,
            ins=[src.ap().opt()], outs=[dst.ap().opt()])

    with tile.TileContext(nc) as tc:
        with tc.tile_pool(name="const", bufs=1) as constp:
            wts = constp.tile([P, KT * Dout], BF16)
            nc.sync.dma_start(out=wts[:], in_=wt_d[:, :])
            iota_t = constp.tile([P, P], F32)
            nc.sync.dma_start(out=iota_t[:], in_=iota_d[:, :])
            ident = constp.tile([P, P], BF16)
            nc.sync.dma_start(out=ident[:], in_=id_d[:, :])
            scl_t = constp.tile([P, 3 * T], F32)
            nc.sync.dma_start(out=scl_t[:], in_=scl_d[:, :])
            idxlo_t = constp.tile([P, NL * 8], I16)
            nc.sync.dma_start(out=idxlo_t[:], in_=idxlo_d[:, :])
            idxhi_t = constp.tile([P, NH * 8], I16)
            nc.sync.dma_start(out=idxhi_t[:], in_=idxhi_d[:, :])
            dllo_t = constp.tile([P, NL], F32)
            nc.sync.dma_start(out=dllo_t[:], in_=dllo_d[:, :])
            dlhi_t = constp.tile([P, NH], F32)
            nc.sync.dma_start(out=dlhi_t[:], in_=dlhi_d[:, :])

            # hop-input locals (row-major, dinv-scaled), SBUF-resident
            tloc = [constp.tile([P, T * Dout], BF16, tag=f"tloc{k}",
                                name=f"tloc{k}")
                    for k in range(2)]

            # ---------------- projection: t0 = dinv * (x @ W.T) ----------
            with tc.tile_pool(name="proj", bufs=1) as projp, \
                 tc.tile_pool(name="ppsum", bufs=4, space="PSUM") as ppsum:
                xts = projp.tile([P, KT * PNp], BF16)
                for k in range(KT):
                    for lo, ln in ((0, RA), (RA, RB)):
                        nc.sync.dma_start(
                            out=xts[:, k * PNp + lo:k * PNp + lo + ln],
                            in_=xt_d[:, k * PNp + lo:k * PNp + lo + ln])
                for m in range(T):
                    ps = ppsum.tile([P, Dout], F32)
                    for k in range(KT):
                        nc.tensor.matmul(
                            out=ps[:, :],
                            lhsT=xts[:,
                                     k * PNp + m * P:k * PNp + (m + 1) * P],
                            rhs=wts[:, k * Dout:(k + 1) * Dout],
                            start=(k == 0), stop=(k == KT - 1))
                    dstv = tloc[0][:, m * Dout:(m + 1) * Dout]
                    nc.vector.tensor_tensor(
                        out=dstv, in0=ps[:, :],
                        in1=scl_t[:, m:m + 1].broadcast_to([P, Dout]),
                        op=mybir.AluOpType.mult)
                    if m < TS:
                        nc.sync.dma_start(
                            out=h0sA[m * P:(m + 1) * P, 0:Dout], in_=dstv)
                    else:
                        nc.sync.dma_start(
                            out=h0sB[m * P - RA:(m + 1) * P - RA, 0:Dout],
                            in_=dstv)
                    if m == TS - 1:
                        allgather(h0sA, h0fA)
                allgather(h0sB, h0fB)

            def hop(kk, tblA, tblB, dst_write):
                with tc.tile_pool(name=f"vals{kk}", bufs=1) as valsp, \
                     tc.tile_pool(name=f"seg{kk}", bufs=4) as segp, \
                     tc.tile_pool(name=f"hp{kk}", bufs=4,
                                  space="PSUM") as hps:
                    DR = 2 * Dout
                    vlo = valsp.tile([P, max(NL, 1) * DR], BF16, tag="vlo",
                                     name="vlo")
                    vhi = valsp.tile([P, max(NH, 1) * DR], BF16, tag="vhi",
                                     name="vhi")
                    qn = 0
                    for vt, nblk, idx_t, tbl in ((vlo, NL, idxlo_t, tblA),
                                                 (vhi, NH, idxhi_t, tblB)):
                        s0 = 0
                        while s0 < nblk:
                            s1 = min(s0 + SEG_CHUNKS, nblk)
                            nb = s1 - s0
                            nc.gpsimd.dma_gather(
                                out_ap=vt[:, s0 * DR:s1 * DR].rearrange(
                                    "p (b f) -> p b f", f=DR),
                                in_ap=tbl[:, :],
                                idxs_ap=idx_t[:, s0 * 8:s1 * 8],
                                num_idxs=nb * P,
                                num_idxs_reg=nb * P,
                                elem_size=DR,
                                queue_num=qn % 4)
                            qn += 1
                            s0 = s1

                    ofs = [0, 0]
                    for t in range(T):
                        nlo = int(nch_u[t, 0])
                        nhi = int(nch_u[t, 1])
                        chunks = ([(0, ofs[0] + i) for i in range(nlo)]
                                  + [(1, ofs[1] + i) for i in range(nhi)])
                        ofs[0] += nlo
                        ofs[1] += nhi
                        ps = hps.tile([P, Dout], F32)
                        nc.tensor.matmul(
                            out=ps[:, :], lhsT=ident[:],
                            rhs=tloc[kk][:, t * Dout:(t + 1) * Dout],
                            start=True, stop=False)
                        for ci, (h, blk) in enumerate(chunks):
                            vt = vlo if h == 0 else vhi
                            dlt = dllo_t if h == 0 else dlhi_t
                            sg = segp.tile([P, P], BF16, name="sg", tag="sg")
                            nc.vector.tensor_tensor(
                                out=sg[:],
                                in0=iota_t[:],
                                in1=dlt[:, blk:blk + 1].broadcast_to(
                                    [P, P]),
                                op=mybir.AluOpType.is_equal)
                            nc.tensor.matmul(
                                out=ps[:, :],
                                lhsT=sg[:],
                                rhs=vt[:, blk * DR:blk * DR + Dout],
                                start=False,
                                stop=(ci == len(chunks) - 1))
                        dst_write(t, ps)

            def hop1_write(t, ps):
                dstv = tloc[1][:, t * Dout:(t + 1) * Dout]
                nc.vector.tensor_tensor(
                    out=dstv, in0=ps[:, :],
                    in1=scl_t[:, T + t:T + t + 1].broadcast_to([P, Dout]),
                    op=mybir.AluOpType.mult)
                if t < TS:
                    nc.sync.dma_start(out=h1sA[t * P:(t + 1) * P, 0:Dout],
                                      in_=dstv)
                    if t == TS - 1:
                        allgather(h1sA, h1fA)
                else:
                    nc.sync.dma_start(
                        out=h1sB[t * P - RA:(t + 1) * P - RA, 0:Dout],
                        in_=dstv)

            hop(0, h0fA, h0fB, hop1_write)
            allgather(h1sB, h1fB)

            with tc.tile_pool(name="outp", bufs=3) as outp:

                def out_write(t, ps):
                    ot = outp.tile([P, Dout], F32, name="ot", tag="ot")
                    nc.vector.tensor_tensor(
                        out=ot[:, :], in0=ps[:, :],
                        in1=scl_t[:, 2 * T + t:2 * T + t + 1].broadcast_to(
                            [P, Dout]),
                        op=mybir.AluOpType.mult)
                    nc.sync.dma_start(out=out_d[t * P:(t + 1) * P, :],
                                      in_=ot[:, :])

                hop(1, h1fA, h1fB, out_write)

    nc.compile()
    return nc


def kernel(**inputs):
    global LAST_RESULTS
    x = inputs["x"]
    W = inputs["W"]
    edge_index = inputs["edge_index"]

    dims, nch_u, in_maps, pos = _prepare(x, edge_index, W)
    nc = _build(dims, nch_u)

    trace = bool(int(os.environ.get("GNN_TRACE", "0")))
    kwargs = {}
    if trace:
        kwargs["trace"] = True
        kwargs["trace_cores"] = list(range(NCORES))
    res = run_bass_kernel_spmd(nc, in_maps, core_ids=list(range(NCORES)),
                               **kwargs)
    LAST_RESULTS = res
    full = np.concatenate(
        [res.results[c]["out"] for c in range(NCORES)], axis=0)
    out = full[pos]
    return np.ascontiguousarray(out, dtype=np.float32)

